# revision 13
# baseline (speedup 1.0000x reference)
"""Causal self-attention (non-masked softmax path) for TRN2, 8 NeuronCores.

Sharding: 2-way data parallel over batch x 4-way tensor parallel over heads.
Core c handles batch b = c // 4, head group g = c % 4 (heads 4g..4g+3).
Host sums the 4 row-parallel c_proj partials per batch and adds b_proj.

v2 design (vs v1): software-pipelined phases + flipped PV.
  - PV orientation: lhsT = E^T chunk [128 keys, 128 queries], rhs = v
    [128 keys, 65] (64 hd + ones column) -> out y [128 queries, 65] with the
    softmax denominator in free column 64.  No output-partition waste
    (v1 used 65 of 128 partitions), and normalization becomes a DVE
    tensor_scalar with a per-partition reciprocal - no selector matmuls.
  - One PV burst (16 matmuls, all 4 heads x 4 query subchunks) per key
    chunk i consumes E slot i right after exp writes it: E ring is small.
  - y [queries, feats] is transposed back to y^T via PE transpose (identity
    matmul) for c_proj, which needs feats on partitions.
  - Everything pipelines at the slot level: projections interleave under
    attention j=0; normalize/transpose/c_proj of chunk j interleave under
    attention j+1; ACT (exp, ~118us) and PE (~111us) are co-critical.

All matmuls bf16 inputs (fp32 matmul = 4x slower), fp32 PSUM accumulate.

PSUM budget (8 banks): tag "s" 2 bufs x [128,1024] f32 (4 banks) shared by
S^T tiles, projection tiles, transpose staging (bitcast bf16) and c_proj;
tag "y" 4 bufs x [128,260] f32 (4 banks) for the per-m PV accumulators.
"""

import numpy as np

B, T, H, NH, HD = 2, 2048, 1024, 16, 64
P = 128
FG = 256          # features per head group (4 heads x 64)
VC = 65           # v columns per head incl. the ones column
NQ = 512          # query chunk (psum free dim)
NJ = T // NQ      # 4
NI = T // P       # 16 key chunks
KH = H // P       # 8 hidden chunks
ESLOTS = NI + 2   # E ring slots (j=0 holds a full chunk's worth + lag)
PVLAG = 2         # PV burst i runs 2 slots behind exp i
NCORES = 8

_CACHE = {}


def _build():
    import concourse.bacc as bacc
    import concourse.mybir as mybir
    import concourse.tile as tile

    fp32 = mybir.dt.float32
    bf16 = mybir.dt.bfloat16

    nc = bacc.Bacc("TRN2", debug=False)
    xT = nc.dram_tensor("xT", [H, T], bf16, kind="ExternalInput").ap()
    wqkv = nc.dram_tensor("wqkv", [H, 3 * FG], bf16, kind="ExternalInput").ap()
    bqk = nc.dram_tensor("bqk", [2 * FG], fp32, kind="ExternalInput").ap()
    bv = nc.dram_tensor("bv", [FG], bf16, kind="ExternalInput").ap()
    wp = nc.dram_tensor("wp", [FG, H], bf16, kind="ExternalInput").ap()
    out = nc.dram_tensor("out", [T, H], fp32, kind="ExternalOutput").ap()

    with tile.TileContext(nc) as tc:
        _emit(nc, tc, mybir, xT, wqkv, bqk, bv, wp, out)
    nc.compile()
    return nc


def _emit(nc, tc, mybir, xT, wqkv, bqk, bv, wp, out):
    from contextlib import ExitStack
    from concourse.masks import make_identity

    fp32 = mybir.dt.float32
    bf16 = mybir.dt.bfloat16
    Exp = mybir.ActivationFunctionType.Exp
    Mult = mybir.AluOpType.mult

    W3 = 3 * FG  # 768, wqkv row width

    with ExitStack() as ctx:
        pool = lambda name, bufs=1, space="SBUF": ctx.enter_context(
            tc.tile_pool(name=name, bufs=bufs, space=space)
        )

        const = pool("const")
        ones = const.tile([1, P], bf16)
        nc.vector.memset(ones[:], 1.0)
        bias0 = const.tile([P, 1], fp32)
        nc.vector.memset(bias0[:], 0.0)
        ident = const.tile([P, P], bf16)
        make_identity(nc, ident[:])
        bqk_sb = const.tile([P, 4], fp32)
        nc.sync.dma_start(bqk_sb[:], bqk.rearrange("(m p) -> p m", p=P))
        bv_sb = const.tile([1, FG], bf16)
        nc.sync.dma_start(bv_sb[:], bv.rearrange("(o f) -> o f", o=1))

        xt_sb = pool("xt").tile([P, KH * T], bf16)
        w_sb = pool("w").tile([P, KH * W3], bf16)
        wp_sb = pool("wp").tile([P, 2 * H], bf16)
        # Chunk-major loads, each chunk split into queue-parallel pieces so
        # chunk k completes ~((k+1)/KH) of the way through the load instead
        # of everything arriving together (DMA queues run pieces in
        # parallel; one big dma_start rides a single queue).  wp last.
        for k in range(KH):
            for c in range(2):
                nc.sync.dma_start(
                    w_sb[:, k * W3 + c * 384 : k * W3 + (c + 1) * 384],
                    wqkv[k * P : (k + 1) * P, c * 384 : (c + 1) * 384],
                )
            for c in range(4):
                nc.sync.dma_start(
                    xt_sb[:, k * T + c * NQ : k * T + (c + 1) * NQ],
                    xT[k * P : (k + 1) * P, c * NQ : (c + 1) * NQ],
                )
        for kk in range(2):
            nc.sync.dma_start(
                wp_sb[:, kk * H : (kk + 1) * H], wp[kk * P : (kk + 1) * P, :]
            )

        qk_sb = pool("qk").tile([P, 4 * T], bf16)   # feat pairs: q0 q1 k0 k1
        v_sb = pool("v").tile([P, NI * 4 * VC], bf16)
        v4 = v_sb.rearrange("p (t h c) -> p t h c", t=NI, h=4, c=VC)
        nc.vector.memset(v4[:, :, :, 64:65], 1.0)   # denominator ones columns
        e_sb = pool("e").tile([P, ESLOTS * 4 * NQ], bf16)

        ps_pool = ctx.enter_context(tc.tile_pool(name="ps", bufs=2, space="PSUM"))
        nrm = pool("nrm", bufs=2)
        ynp = pool("yn", bufs=2)
        ytp = pool("yt", bufs=2)
        outp = pool("outp", bufs=3)

        # ---------- emission helpers ----------
        def proj_qk(m, jj):
            ps = ps_pool.tile([P, 2 * NQ], fp32, tag="s", name=f"pqk{m}_{jj}")
            for k in range(KH):
                nc.tensor.matmul(
                    ps[:, 0:NQ],
                    w_sb[:, k * W3 + m * P : k * W3 + (m + 1) * P],
                    xt_sb[:, k * T + jj * NQ : k * T + (jj + 1) * NQ],
                    start=(k == 0),
                    stop=(k == KH - 1),
                )
            # psum -> bf16 qk_sb with per-partition bias, on DVE (ACT is
            # reserved for exp)
            nc.vector.tensor_scalar(
                qk_sb[:, m * T + jj * NQ : m * T + (jj + 1) * NQ],
                ps[:, 0:NQ],
                bqk_sb[:, m : m + 1],
                None,
                mybir.AluOpType.add,
            )

        def proj_v(t):
            ps = ps_pool.tile([P, 2 * NQ], fp32, tag="s", name=f"pv{t}")
            for k in range(KH):
                nc.tensor.matmul(
                    ps[:, 0:FG],
                    xt_sb[:, k * T + t * P : k * T + (t + 1) * P],
                    w_sb[:, k * W3 + 2 * FG : (k + 1) * W3],
                    start=(k == 0),
                    stop=False,
                )
            nc.tensor.matmul(  # += ones.T @ bv (bias broadcast over rows)
                ps[:, 0:FG],
                ones[0:1, :],
                bv_sb[0:1, :],
                start=False,
                stop=True,
            )
            nc.vector.tensor_copy(
                v4[:, t, :, 0:64],
                ps[:, 0:FG].rearrange("p (h c) -> p h c", h=4, c=64),
            )

        def s_pair(j, i, p):
            sp = ps_pool.tile([P, 2 * NQ], fp32, tag="s", name=f"s{j}_{i}_{p}")
            for hh in range(2):
                bp = 64 * hh
                nc.tensor.matmul(
                    sp[:, hh * NQ : (hh + 1) * NQ],
                    qk_sb[
                        bp : bp + 64,
                        (2 + p) * T + i * P : (2 + p) * T + (i + 1) * P,
                    ],
                    qk_sb[bp : bp + 64, p * T + j * NQ : p * T + (j + 1) * NQ],
                    start=True,
                    stop=True,
                    tile_position=(bp, 0),
                )
            return sp

        def exp_pair(j, i, p, sp):
            base = ((j * NI + i) % ESLOTS) * 4 * NQ + 2 * p * NQ
            nc.scalar.activation(
                e_sb[:, base : base + 2 * NQ], sp[:], Exp, bias=bias0[:, 0:1]
            )

        def pv_burst(j, i, yps):
            sbase = ((j * NI + i) % ESLOTS) * 4 * NQ
            for m in range(4):
                for h in range(4):
                    eb = sbase + h * NQ + m * P
                    # One accumulation group per PSUM bank (zero regions are
                    # bank-granular): start zeroes the whole bank lazily, the
                    # other heads' first writes land on zeroed bytes.
                    nc.tensor.matmul(
                        yps[m][:, h * VC : (h + 1) * VC],
                        e_sb[:, eb : eb + P],
                        v4[:, i, h, :],
                        start=(i == 0 and h == 0),
                        stop=(i == NI - 1 and h == 3),
                        skip_group_check=True,
                    )

        def norm(j, m, yps_m):
            ypr = yps_m.rearrange("p (h c) -> p h c", h=4, c=VC)
            rcp = nrm.tile([P, 4], fp32, tag="rcp", name=f"rcp{j}_{m}")
            rcpv = rcp.rearrange("p (h o) -> p h o", h=4, o=1)
            nc.vector.reciprocal(rcpv[:], ypr[:, :, 64:65])
            yn = ynp.tile([P, FG], bf16, tag="yn", name=f"yn{j}_{m}")
            for h in range(4):
                nc.vector.tensor_scalar(
                    yn[:, h * 64 : (h + 1) * 64],
                    ypr[:, h, 0:64],
                    rcp[:, h : h + 1],
                    None,
                    Mult,
                )
            return yn

        def transp(j, m, yn):
            tp = ps_pool.tile([P, 2 * NQ], fp32, tag="s", name=f"tp{j}_{m}")
            tpb = tp.bitcast(bf16)
            for kk in range(2):
                # separate PSUM banks: each transpose is its own
                # (bank-granular) accumulation group
                nc.tensor.transpose(
                    tpb[:, kk * 2 * NQ : kk * 2 * NQ + P],
                    yn[:, kk * P : (kk + 1) * P],
                    ident[:],
                )
            yt = ytp.tile([P, 2 * P], bf16, tag="yt", name=f"yt{j}_{m}")
            ytv = yt.rearrange("p (k t) -> p k t", k=2, t=P)
            nc.vector.tensor_copy(
                ytv[:], tpb[:].rearrange("p (k t) -> p k t", k=2, t=2 * NQ)[:, :, 0:P]
            )
            return yt

        def cproj(j, m, yt):
            cp = ps_pool.tile([P, 2 * NQ], fp32, tag="s", name=f"cp{j}_{m}")
            for n in range(2):
                for kk in range(2):
                    nc.tensor.matmul(
                        cp[:, n * NQ : (n + 1) * NQ],
                        yt[:, kk * P : (kk + 1) * P],
                        wp_sb[:, kk * H + n * NQ : kk * H + (n + 1) * NQ],
                        start=(kk == 0),
                        stop=(kk == 1),
                    )
            for n in range(2):
                ot = outp.tile([P, NQ], fp32, tag="o", name=f"o{j}_{m}_{n}")
                nc.vector.tensor_copy(ot[:], cp[:, n * NQ : (n + 1) * NQ])
                r0 = j * NQ + m * P
                nc.sync.dma_start(out[r0 : r0 + P, n * NQ : (n + 1) * NQ], ot[:])

        # ---------- emission schedule ----------
        # One slot = S-pair + exp for (j, i, p), emitted as groups g(j,p)
        # with i = 0..15.  Background PE work (projection units, PV bursts,
        # drains) is spread so no group's PE work exceeds its ACT time
        # (~17us per 16-exp group); emission order is program order, so
        # every data producer is emitted before its consumer, and PV bursts
        # precede the exp of the same slot (the exp may recycle the E-ring
        # slot the burst reads).
        #
        # g(0,p0): S/exp(0,p0) + [k0jj1-3, k1, q1jj0] at 1 unit per 2 slots
        # g(0,p1): S/exp(0,p1) + v(0..10) + q0jj1 + q1jj1
        # g(1,p0): S/exp(1,p0) + v(11..15) + PV(0) at 2 bursts/slot + norms(0)
        # g(1,p1): S/exp(1,p1) + PV(1) lag 2 + transp/cproj(0) + q0jj2,q1jj2
        # g(j,p0) j=2,3: S/exp + norms(j-1) + transp/cproj(j-1) + qjj(j+1)
        # g(j,p1) j=2,3: S/exp + PV(j) lag 2
        # tail: norms(3) + transp/cproj(3)
        proj_qk(2, 0)
        proj_qk(0, 0)
        yps = {}
        yn = {}
        yt = {}

        # ---- g(0, p0)
        bg0 = [(2, 1), (3, 0), (2, 2), (3, 1), (2, 3), (3, 2), (3, 3), (1, 0)]
        for i in range(NI):
            sp = s_pair(0, i, 0)
            exp_pair(0, i, 0, sp)
            if i % 2 == 0:
                proj_qk(*bg0[i // 2])
        # ---- g(0, p1)
        for i in range(NI):
            sp = s_pair(0, i, 1)
            exp_pair(0, i, 1, sp)
            if i < 11:
                proj_v(i)
            elif i == 11:
                proj_qk(0, 1)
            elif i == 12:
                proj_qk(1, 1)
        # ---- g(1, p0)
        yps[0] = [
            ps_pool.tile([P, 4 * VC], fp32, tag="y", bufs=4, name=f"y0_{m}")
            for m in range(4)
        ]
        for i in range(NI):
            sp = s_pair(1, i, 0)
            if i >= 2:
                for b in (2 * (i - 2), 2 * (i - 2) + 1):
                    if b < NI:
                        pv_burst(0, b, yps[0])
            exp_pair(1, i, 0, sp)
            if i < 5:
                proj_v(11 + i)
            elif 10 <= i < 14:
                m = i - 10
                yn[m] = norm(0, m, yps[0][m])
        # ---- g(1, p1)
        yps[1] = [
            ps_pool.tile([P, 4 * VC], fp32, tag="y", bufs=4, name=f"y1_{m}")
            for m in range(4)
        ]
        for i in range(NI):
            sp = s_pair(1, i, 1)
            if i >= PVLAG:
                pv_burst(1, i - PVLAG, yps[1])
            exp_pair(1, i, 1, sp)
            if i in (3, 6, 9, 12):
                m = i // 3 - 1
                yt[m] = transp(0, m, yn[m])
            if i in (5, 8, 11, 14):
                m = (i - 5) // 3
                cproj(0, m, yt[m])
            if i == 0:
                proj_qk(0, 2)
            elif i == 1:
                proj_qk(1, 2)
        for i in range(NI - PVLAG, NI):
            pv_burst(1, i, yps[1])

        # ---- g(j, *) for j = 2, 3
        for j in (2, 3):
            for i in range(NI):
                sp = s_pair(j, i, 0)
                exp_pair(j, i, 0, sp)
                if i < 4:
                    m = i
                    yn[m] = norm(j - 1, m, yps[j - 1][m])
                elif i in (5, 7, 9, 11):
                    m = (i - 5) // 2
                    yt[m] = transp(j - 1, m, yn[m])
                elif i in (6, 8, 10, 12):
                    m = (i - 6) // 2
                    cproj(j - 1, m, yt[m])
                elif i == 13 and j == 2:
                    proj_qk(0, 3)
                elif i == 14 and j == 2:
                    proj_qk(1, 3)
            yps[j] = [
                ps_pool.tile([P, 4 * VC], fp32, tag="y", bufs=4, name=f"y{j}_{m}")
                for m in range(4)
            ]
            for i in range(NI):
                sp = s_pair(j, i, 1)
                if i >= PVLAG:
                    pv_burst(j, i - PVLAG, yps[j])
                exp_pair(j, i, 1, sp)
            for i in range(NI - PVLAG, NI):
                pv_burst(j, i, yps[j])

        # tail: drain j=3
        for m in range(4):
            yn3 = norm(NJ - 1, m, yps[NJ - 1][m])
            yt3 = transp(NJ - 1, m, yn3)
            cproj(NJ - 1, m, yt3)


def _get_nc():
    if "nc" not in _CACHE:
        _CACHE["nc"] = _build()
    return _CACHE["nc"]


def _make_in_maps(x, W_attn, b_attn, W_proj):
    import ml_dtypes

    bf = ml_dtypes.bfloat16
    x = np.asarray(x, np.float32)
    W_attn = np.asarray(W_attn, np.float32)
    b_attn = np.asarray(b_attn, np.float32)
    W_proj = np.asarray(W_proj, np.float32)
    scale = 1.0 / np.sqrt(np.float32(HD))
    in_maps = []
    for c in range(NCORES):
        b, g = divmod(c, 4)
        sl = slice(FG * g, FG * (g + 1))
        wq = W_attn[:, sl] * scale
        wk = W_attn[:, H:][:, sl]
        wv = W_attn[:, 2 * H :][:, sl]
        in_maps.append(
            {
                "xT": np.ascontiguousarray(x[b].T).astype(bf),
                "wqkv": np.ascontiguousarray(
                    np.concatenate([wq, wk, wv], axis=1)
                ).astype(bf),
                "bqk": np.concatenate(
                    [b_attn[sl] * scale, b_attn[H:][sl]]
                ).astype(np.float32),
                "bv": np.ascontiguousarray(b_attn[2 * H :][sl]).astype(bf),
                "wp": np.ascontiguousarray(W_proj[sl, :]).astype(bf),
            }
        )
    return in_maps


def _gather(results, b_proj):
    b_proj = np.asarray(b_proj, np.float32)
    y = np.empty((B, T, H), np.float32)
    for b in range(B):
        acc = results[4 * b]["out"].astype(np.float32)
        for g in range(1, 4):
            acc = acc + results[4 * b + g]["out"]
        y[b] = acc + b_proj[None, :]
    return y


def run(x, W_attn, b_attn, W_proj, b_proj, trace=False):
    from concourse.bass_utils import run_bass_kernel_spmd

    nc = _get_nc()
    in_maps = _make_in_maps(x, W_attn, b_attn, W_proj)
    res = run_bass_kernel_spmd(nc, in_maps, list(range(NCORES)), trace=trace)
    return _gather(res.results, b_proj), res


def kernel(x, W_attn, b_attn, W_proj, b_proj):
    y, _ = run(x, W_attn, b_attn, W_proj, b_proj, trace=False)
    return y


# revision 15
# speedup vs baseline: 1.0532x; 1.0532x over previous
"""Causal self-attention (non-masked softmax path) for TRN2, 8 NeuronCores.

Sharding: 2-way data parallel over batch x 4-way tensor parallel over heads.
Core c handles batch b = c // 4, head group g = c % 4 (heads 4g..4g+3).
Host sums the 4 row-parallel c_proj partials per batch and adds b_proj.

v2 design (vs v1): software-pipelined phases + flipped PV.
  - PV orientation: lhsT = E^T chunk [128 keys, 128 queries], rhs = v
    [128 keys, 65] (64 hd + ones column) -> out y [128 queries, 65] with the
    softmax denominator in free column 64.  No output-partition waste
    (v1 used 65 of 128 partitions), and normalization becomes a DVE
    tensor_scalar with a per-partition reciprocal - no selector matmuls.
  - One PV burst (16 matmuls, all 4 heads x 4 query subchunks) per key
    chunk i consumes E slot i right after exp writes it: E ring is small.
  - y [queries, feats] is transposed back to y^T via PE transpose (identity
    matmul) for c_proj, which needs feats on partitions.
  - Everything pipelines at the slot level: projections interleave under
    attention j=0; normalize/transpose/c_proj of chunk j interleave under
    attention j+1; ACT (exp, ~118us) and PE (~111us) are co-critical.

All matmuls bf16 inputs (fp32 matmul = 4x slower), fp32 PSUM accumulate.

PSUM budget (8 banks): tag "s" 2 bufs x [128,1024] f32 (4 banks) shared by
S^T tiles, projection tiles, transpose staging (bitcast bf16) and c_proj;
tag "y" 4 bufs x [128,260] f32 (4 banks) for the per-m PV accumulators.
"""

import numpy as np

B, T, H, NH, HD = 2, 2048, 1024, 16, 64
P = 128
FG = 256          # features per head group (4 heads x 64)
VC = 65           # v columns per head incl. the ones column
NQ = 512          # query chunk (psum free dim)
NJ = T // NQ      # 4
NI = T // P       # 16 key chunks
KH = H // P       # 8 hidden chunks
ESLOTS = NI + 2   # E ring slots (j=0 holds a full chunk's worth + lag)
PVLAG = 2         # PV burst i runs 2 slots behind exp i
NCORES = 8

_CACHE = {}


def _build():
    import concourse.bacc as bacc
    import concourse.mybir as mybir
    import concourse.tile as tile

    fp32 = mybir.dt.float32
    bf16 = mybir.dt.bfloat16

    nc = bacc.Bacc("TRN2", debug=False)
    xT = nc.dram_tensor("xT", [H, T], bf16, kind="ExternalInput").ap()
    wqkv = nc.dram_tensor("wqkv", [H, 3 * FG], bf16, kind="ExternalInput").ap()
    bqk = nc.dram_tensor("bqk", [2 * FG], fp32, kind="ExternalInput").ap()
    bv = nc.dram_tensor("bv", [FG], bf16, kind="ExternalInput").ap()
    wp = nc.dram_tensor("wp", [FG, H], bf16, kind="ExternalInput").ap()
    out = nc.dram_tensor("out", [T, H], bf16, kind="ExternalOutput").ap()

    with tile.TileContext(nc) as tc:
        _emit(nc, tc, mybir, xT, wqkv, bqk, bv, wp, out)
    nc.compile()
    return nc


def _emit(nc, tc, mybir, xT, wqkv, bqk, bv, wp, out):
    from contextlib import ExitStack
    from concourse.masks import make_identity

    fp32 = mybir.dt.float32
    bf16 = mybir.dt.bfloat16
    Exp = mybir.ActivationFunctionType.Exp
    Mult = mybir.AluOpType.mult

    W3 = 3 * FG  # 768, wqkv row width

    with ExitStack() as ctx:
        pool = lambda name, bufs=1, space="SBUF": ctx.enter_context(
            tc.tile_pool(name=name, bufs=bufs, space=space)
        )

        const = pool("const")
        ones = const.tile([1, P], bf16)
        nc.vector.memset(ones[:], 1.0)
        bias0 = const.tile([P, 1], fp32)
        nc.vector.memset(bias0[:], 0.0)
        ident = const.tile([P, P], bf16)
        make_identity(nc, ident[:])
        bqk_sb = const.tile([P, 4], fp32)
        nc.sync.dma_start(bqk_sb[:], bqk.rearrange("(m p) -> p m", p=P))
        bv_sb = const.tile([1, FG], bf16)
        nc.sync.dma_start(bv_sb[:], bv.rearrange("(o f) -> o f", o=1))

        xt_sb = pool("xt").tile([P, KH * T], bf16)
        w_sb = pool("w").tile([P, KH * W3], bf16)
        wp_sb = pool("wp").tile([P, 2 * H], bf16)
        # One big transfer per chunk (16 transfers fill the 16 queues and
        # share bandwidth evenly); the projections need all chunks anyway,
        # so what matters is total load time, not per-chunk arrival.
        for k in range(KH):
            nc.sync.dma_start(xt_sb[:, k * T : (k + 1) * T], xT[k * P : (k + 1) * P, :])
            nc.sync.dma_start(
                w_sb[:, k * W3 : (k + 1) * W3], wqkv[k * P : (k + 1) * P, :]
            )
        for kk in range(2):
            nc.sync.dma_start(
                wp_sb[:, kk * H : (kk + 1) * H], wp[kk * P : (kk + 1) * P, :]
            )

        qk_sb = pool("qk").tile([P, 4 * T], bf16)   # feat pairs: q0 q1 k0 k1
        v_sb = pool("v").tile([P, NI * 4 * VC], bf16)
        v4 = v_sb.rearrange("p (t h c) -> p t h c", t=NI, h=4, c=VC)
        nc.vector.memset(v4[:, :, :, 64:65], 1.0)   # denominator ones columns
        e_sb = pool("e").tile([P, ESLOTS * 4 * NQ], bf16)

        ps_pool = ctx.enter_context(tc.tile_pool(name="ps", bufs=2, space="PSUM"))
        nrm = pool("nrm", bufs=2)
        ynp = pool("yn", bufs=2)
        ytp = pool("yt", bufs=2)
        outp = pool("outp", bufs=3)

        # ---------- emission helpers ----------
        def proj_qk(m, jj):
            ps = ps_pool.tile([P, 2 * NQ], fp32, tag="s", name=f"pqk{m}_{jj}")
            for k in range(KH):
                nc.tensor.matmul(
                    ps[:, 0:NQ],
                    w_sb[:, k * W3 + m * P : k * W3 + (m + 1) * P],
                    xt_sb[:, k * T + jj * NQ : k * T + (jj + 1) * NQ],
                    start=(k == 0),
                    stop=(k == KH - 1),
                )
            # psum -> bf16 qk_sb with per-partition bias, on DVE (ACT is
            # reserved for exp)
            nc.vector.tensor_scalar(
                qk_sb[:, m * T + jj * NQ : m * T + (jj + 1) * NQ],
                ps[:, 0:NQ],
                bqk_sb[:, m : m + 1],
                None,
                mybir.AluOpType.add,
            )

        def proj_v(t):
            ps = ps_pool.tile([P, 2 * NQ], fp32, tag="s", name=f"pv{t}")
            for k in range(KH):
                nc.tensor.matmul(
                    ps[:, 0:FG],
                    xt_sb[:, k * T + t * P : k * T + (t + 1) * P],
                    w_sb[:, k * W3 + 2 * FG : (k + 1) * W3],
                    start=(k == 0),
                    stop=False,
                )
            nc.tensor.matmul(  # += ones.T @ bv (bias broadcast over rows)
                ps[:, 0:FG],
                ones[0:1, :],
                bv_sb[0:1, :],
                start=False,
                stop=True,
            )
            nc.vector.tensor_copy(
                v4[:, t, :, 0:64],
                ps[:, 0:FG].rearrange("p (h c) -> p h c", h=4, c=64),
            )

        def s_pair(j, i, p):
            sp = ps_pool.tile([P, 2 * NQ], fp32, tag="s", name=f"s{j}_{i}_{p}")
            for hh in range(2):
                bp = 64 * hh
                nc.tensor.matmul(
                    sp[:, hh * NQ : (hh + 1) * NQ],
                    qk_sb[
                        bp : bp + 64,
                        (2 + p) * T + i * P : (2 + p) * T + (i + 1) * P,
                    ],
                    qk_sb[bp : bp + 64, p * T + j * NQ : p * T + (j + 1) * NQ],
                    start=True,
                    stop=True,
                    tile_position=(bp, 0),
                )
            return sp

        def exp_pair(j, i, p, sp):
            base = ((j * NI + i) % ESLOTS) * 4 * NQ + 2 * p * NQ
            nc.scalar.activation(
                e_sb[:, base : base + 2 * NQ], sp[:], Exp, bias=bias0[:, 0:1]
            )

        def pv_burst(j, i, yps):
            sbase = ((j * NI + i) % ESLOTS) * 4 * NQ
            for m in range(4):
                for h in range(4):
                    eb = sbase + h * NQ + m * P
                    # One accumulation group per PSUM bank (zero regions are
                    # bank-granular): start zeroes the whole bank lazily, the
                    # other heads' first writes land on zeroed bytes.
                    nc.tensor.matmul(
                        yps[m][:, h * VC : (h + 1) * VC],
                        e_sb[:, eb : eb + P],
                        v4[:, i, h, :],
                        start=(i == 0 and h == 0),
                        stop=(i == NI - 1 and h == 3),
                        skip_group_check=True,
                    )

        def norm(j, m, yps_m):
            ypr = yps_m.rearrange("p (h c) -> p h c", h=4, c=VC)
            rcp = nrm.tile([P, 4], fp32, tag="rcp", name=f"rcp{j}_{m}")
            rcpv = rcp.rearrange("p (h o) -> p h o", h=4, o=1)
            nc.vector.reciprocal(rcpv[:], ypr[:, :, 64:65])
            yn = ynp.tile([P, FG], bf16, tag="yn", name=f"yn{j}_{m}")
            for h in range(4):
                nc.vector.tensor_scalar(
                    yn[:, h * 64 : (h + 1) * 64],
                    ypr[:, h, 0:64],
                    rcp[:, h : h + 1],
                    None,
                    Mult,
                )
            return yn

        def transp(j, m, yn):
            tp = ps_pool.tile([P, 2 * NQ], fp32, tag="s", name=f"tp{j}_{m}")
            tpb = tp.bitcast(bf16)
            for kk in range(2):
                # separate PSUM banks: each transpose is its own
                # (bank-granular) accumulation group
                nc.tensor.transpose(
                    tpb[:, kk * 2 * NQ : kk * 2 * NQ + P],
                    yn[:, kk * P : (kk + 1) * P],
                    ident[:],
                )
            yt = ytp.tile([P, 2 * P], bf16, tag="yt", name=f"yt{j}_{m}")
            ytv = yt.rearrange("p (k t) -> p k t", k=2, t=P)
            nc.vector.tensor_copy(
                ytv[:], tpb[:].rearrange("p (k t) -> p k t", k=2, t=2 * NQ)[:, :, 0:P]
            )
            return yt

        def cproj(j, m, yt):
            cp = ps_pool.tile([P, 2 * NQ], fp32, tag="s", name=f"cp{j}_{m}")
            for n in range(2):
                for kk in range(2):
                    nc.tensor.matmul(
                        cp[:, n * NQ : (n + 1) * NQ],
                        yt[:, kk * P : (kk + 1) * P],
                        wp_sb[:, kk * H + n * NQ : kk * H + (n + 1) * NQ],
                        start=(kk == 0),
                        stop=(kk == 1),
                    )
            for n in range(2):
                ot = outp.tile([P, NQ], bf16, tag="o", name=f"o{j}_{m}_{n}")
                nc.vector.tensor_copy(ot[:], cp[:, n * NQ : (n + 1) * NQ])
                r0 = j * NQ + m * P
                nc.sync.dma_start(out[r0 : r0 + P, n * NQ : (n + 1) * NQ], ot[:])

        # ---------- emission schedule ----------
        # One slot = S-pair + exp for (j, i, p); groups g(j,p) with i=0..15.
        # g(0,p0) emits its 16 S/exp back-to-back (they only need k0+q0jj0
        # from the pre units) so ACT saturates as soon as the x/w DMA + k0
        # projection land; all other projection units are emitted AFTER the
        # S-pairs they must not block (emission order is PE program order,
        # and a DMA-gated projection at the queue head starves ACT).
        # Drains (transpose/c_proj of j-1) run in g(j,p1) slots where the
        # "s" psum ring has no other background users, so their long slot
        # lifetimes don't delay S-pair allocations.  norms(j-1) right after
        # PV(j-1) tail bursts.  PV(j,i) runs at g(j,p1) slot i+PVLAG.
        for m, jj in [(2, 0), (2, 1), (2, 2), (2, 3), (0, 0)]:
            proj_qk(m, jj)

        yps = {}
        yn = {}
        yt = {}
        for j in range(NJ):
            # ---- g(j, p0)
            if j >= 1:
                for m in range(4):
                    yn[m] = norm(j - 1, m, yps[j - 1][m])
            for i in range(NI):
                sp = s_pair(j, i, 0)
                exp_pair(j, i, 0, sp)
            if j == 0:
                for m, jj in [(3, 0), (3, 1), (3, 2), (3, 3), (1, 0)]:
                    proj_qk(m, jj)
            # ---- g(j, p1)
            yps[j] = [
                ps_pool.tile([P, 4 * VC], fp32, tag="y", bufs=4, name=f"y{j}_{m}")
                for m in range(4)
            ]
            for i in range(NI):
                sp = s_pair(j, i, 1)
                exp_pair(j, i, 1, sp)
                if j == 0:
                    proj_v(i)
                if i >= PVLAG:
                    pv_burst(j, i - PVLAG, yps[j])
                if j >= 1:
                    if i in (5, 8, 11, 14):
                        m = (i - 5) // 3
                        yt[m] = transp(j - 1, m, yn[m])
                    if i in (6, 9, 12, 15):
                        m = (i - 6) // 3
                        cproj(j - 1, m, yt[m])
            for i in range(NI - PVLAG, NI):
                pv_burst(j, i, yps[j])
            # q projections for the next j, after this group's S-pairs
            if j < NJ - 1:
                proj_qk(0, j + 1)
                proj_qk(1, j + 1)

        # tail: drain j=3
        for m in range(4):
            yn3 = norm(NJ - 1, m, yps[NJ - 1][m])
            yt3 = transp(NJ - 1, m, yn3)
            cproj(NJ - 1, m, yt3)


def _get_nc():
    if "nc" not in _CACHE:
        _CACHE["nc"] = _build()
    return _CACHE["nc"]


def _make_in_maps(x, W_attn, b_attn, W_proj):
    import ml_dtypes

    bf = ml_dtypes.bfloat16
    x = np.asarray(x, np.float32)
    W_attn = np.asarray(W_attn, np.float32)
    b_attn = np.asarray(b_attn, np.float32)
    W_proj = np.asarray(W_proj, np.float32)
    scale = 1.0 / np.sqrt(np.float32(HD))
    in_maps = []
    for c in range(NCORES):
        b, g = divmod(c, 4)
        sl = slice(FG * g, FG * (g + 1))
        wq = W_attn[:, sl] * scale
        wk = W_attn[:, H:][:, sl]
        wv = W_attn[:, 2 * H :][:, sl]
        in_maps.append(
            {
                "xT": np.ascontiguousarray(x[b].T).astype(bf),
                "wqkv": np.ascontiguousarray(
                    np.concatenate([wq, wk, wv], axis=1)
                ).astype(bf),
                "bqk": np.concatenate(
                    [b_attn[sl] * scale, b_attn[H:][sl]]
                ).astype(np.float32),
                "bv": np.ascontiguousarray(b_attn[2 * H :][sl]).astype(bf),
                "wp": np.ascontiguousarray(W_proj[sl, :]).astype(bf),
            }
        )
    return in_maps


def _gather(results, b_proj):
    b_proj = np.asarray(b_proj, np.float32)
    y = np.empty((B, T, H), np.float32)
    for b in range(B):
        acc = results[4 * b]["out"].astype(np.float32)
        for g in range(1, 4):
            acc = acc + results[4 * b + g]["out"]
        y[b] = acc + b_proj[None, :]
    return y


def run(x, W_attn, b_attn, W_proj, b_proj, trace=False):
    from concourse.bass_utils import run_bass_kernel_spmd

    nc = _get_nc()
    in_maps = _make_in_maps(x, W_attn, b_attn, W_proj)
    res = run_bass_kernel_spmd(nc, in_maps, list(range(NCORES)), trace=trace)
    return _gather(res.results, b_proj), res


def kernel(x, W_attn, b_attn, W_proj, b_proj):
    y, _ = run(x, W_attn, b_attn, W_proj, b_proj, trace=False)
    return y
